# revision 1
# baseline (speedup 1.0000x reference)
"""DoubleAttention (Performer global heads + local windowed heads) on 8
Trainium2 NeuronCores via Bass/Tile SPMD.

Sharding: 16 (batch, head-pair) jobs over 8 cores with one SPMD program
branching on partition id:
  cores 0-5 : b = c//3, one pair of the 6 global Performer heads
  cores 6,7 : b = c-6,  both local windowed heads
Each core computes q/k/v projections for its 2 heads, its attention, and
a partial output projection y_c = attn_out @ Wo[slice,:]. The host sums
the 4 partials per batch (+bo). Zero cross-core communication.

Exact math restructurings vs the reference (fp-rounding-level equal):
 - Performer `ratio` cancels between numerator and denominator.
 - dd - diag fused into ONE matmul: contract [qT*norm ; qT^2*0.5norm^2]
   (128 rows) against [projT ; -ones].
 - per-query max only affects the eps floor: exp(z-m)+eps is a per-token
   scale (cancels) times exp(z) + eps*e^{m}; the latter enters num/den
   as an appended rank-1 term.
 - k-side global max applied post-hoc: ctx = e^{-m_k}*sum(exp(z_k)[v|1])
   + eps*[vsum|N].
 - local attention: softmax max-subtraction dropped (dots are O(5), exp
   safe in fp32; softmax is shift-invariant). P = exp(dots/8) computed
   KEY-major; [v|1] folds the row-sum into the same A@V matmul.

All heavy matmuls run as float32r (~2e-4 rel err, same order as this
PE's fp32 mode, at 4x throughput). Engines cannot shift partitions
(lane-locked), so the few cross-partition moves go through SBUF->SBUF
DMAs or PE transposes.
"""
import numpy as np
from contextlib import ExitStack

import concourse.bass as bass
import concourse.mybir as mybir
import concourse.tile as tile
from concourse.bass_utils import run_bass_kernel_spmd
from concourse.masks import make_identity

F32 = mybir.dt.float32
F32R = mybir.dt.float32r
AF = mybir.ActivationFunctionType
ALU = mybir.AluOpType
AX = mybir.AxisListType

DM = 512
DH = 64
NF = 256
WIN = 256
EPS = 1e-4
NORM = DH ** -0.25
SQRT_C2 = (0.5 * NORM * NORM) ** 0.5    # Square(x*s) = x^2 * 0.5norm^2

# ---------------------------------------------------------------------------
# walrus wait legalizer: this toolchain's walrus accepts only ONE sync wait
# per instruction; Tile attaches several. Split extras onto NoOps.
# ---------------------------------------------------------------------------
_WNOP = [0]


def _nop(engine, debug, waits=(), updates=()):
    _WNOP[0] += 1
    return {
        "name": f"WNOP-{_WNOP[0]}",
        "opcode": "NoOp",
        "engine": engine,
        "ins": [],
        "outs": [],
        "debug": debug,
        "sync_info": {"on_update": list(updates), "on_wait": list(waits)},
    }


def _legalize_bir_waits(bir_bytes: bytes) -> bytes:
    """Engine instruction structs accept ONE wait and ONE sem-inc(+1)
    update. Tile emits several waits per instruction and (at If-arm clock
    merges) big sem-add-imm updates. Split extras onto NoOps on the same
    queue (engines retire in order, so ordering semantics hold)."""
    import orjson
    d = orjson.loads(bir_bytes)
    for fn in d["functions"]:
        for bb in fn["blocks"]:
            out = []
            for inst in bb["instructions"]:
                op = inst.get("opcode", "")
                si = inst.get("sync_info")
                if si is None or "Branch" in op:
                    out.append(inst)
                    continue
                dbg = inst.get("debug")
                eng = inst["engine"]
                if "DMA" not in op.upper() and op != "ISA":
                    # A big add-imm comes from If-arm clock alignment: v-1
                    # virtual ticks (guarding no data) + this instruction's
                    # own completion. Emit the padding as +1 NoOps BEFORE
                    # the instruction (a trailing NoOp would fire at issue,
                    # before the writes drain) and keep +1 on it.
                    ups = si.get("on_update") or []
                    new_ups = []
                    for u in ups:
                        if (u.get("sync_type") == "semaphore"
                                and u.get("update_mode") in ("sem-inc",
                                                             "sem-add-imm")
                                and int(u.get("update_value", 1)) > 1):
                            v = int(u["update_value"])
                            out.append(_nop(eng, dbg, updates=[
                                dict(u, update_mode="sem-add-imm",
                                     update_value=v - 1)]))
                            new_ups.append(dict(u, update_mode="sem-inc",
                                                update_value=1))
                        else:
                            new_ups.append(u)
                    si["on_update"] = new_ups
                waits = si.get("on_wait") or []
                if len(waits) > 1:
                    for w in waits[:-1]:
                        out.append(_nop(eng, dbg, waits=[w]))
                    si["on_wait"] = [waits[-1]]
                out.append(inst)
            bb["instructions"] = out
    return orjson.dumps(d)


def _install_legalizer():
    import concourse.bass2jax as b2j
    if getattr(b2j, "_wait_legalizer_installed", False):
        return
    orig = b2j.compile_bir_kernel

    def patched(ant_bir_str, *args, **kwargs):
        return orig(_legalize_bir_waits(ant_bir_str), *args, **kwargs)

    b2j.compile_bir_kernel = patched
    b2j._wait_legalizer_installed = True


# ---------------------------------------------------------------------------
# program builder
# ---------------------------------------------------------------------------

class _Env:
    pass


def build_program(n_tok: int, debug: bool = False) -> bass.Bass:
    TT = 512
    NST = n_tok // TT
    NC = n_tok // 128
    NW = n_tok // WIN

    nc = bass.Bass()
    e = _Env()
    e.n_tok, e.TT, e.NST, e.NC, e.NW = n_tok, TT, NST, NC, NW
    e.xT = nc.declare_dram_parameter("xT", [DM, n_tok], F32, isOutput=False)
    e.wq = nc.declare_dram_parameter("wq", [DM, 128], F32, isOutput=False)
    e.wk = nc.declare_dram_parameter("wk", [DM, 128], F32, isOutput=False)
    e.wv = nc.declare_dram_parameter("wv", [DM, 128], F32, isOutput=False)
    e.wo = nc.declare_dram_parameter("wo", [128, DM], F32, isOutput=False)
    e.projT = nc.declare_dram_parameter("projT", [DH, NF], F32, isOutput=False)
    e.cosT = nc.declare_dram_parameter("cosT", [DH, n_tok], F32, isOutput=False)
    e.sinNT = nc.declare_dram_parameter("sinNT", [DH, n_tok], F32, isOutput=False)
    e.y = nc.declare_dram_parameter("y", [n_tok, DM], F32, isOutput=True)
    e.debug = debug
    if debug:
        e.dbgAq = nc.declare_dram_parameter("dbgAq", [128, 512], F32, isOutput=True)
        e.dbgzk = nc.declare_dram_parameter("dbgzk", [128, 256], F32, isOutput=True)
        e.dbgkp = nc.declare_dram_parameter("dbgkp", [128, 256], F32, isOutput=True)
        e.dbgctx = nc.declare_dram_parameter("dbgctx", [65, 256], F32, isOutput=True)
        e.dbgmeta = nc.declare_dram_parameter("dbgmeta", [1, 8], F32, isOutput=True)
        e.dbgctxT = nc.declare_dram_parameter("dbgctxT", [65, 256], F32, isOutput=True)
        e.dbgcfm = nc.declare_dram_parameter("dbgcfm", [128, 2, 68], F32, isOutput=True)
        e.dbgsrow = nc.declare_dram_parameter("dbgsrow", [1, 68], F32, isOutput=True)
        e.dbgqp = nc.declare_dram_parameter("dbgqp", [128, 512], F32, isOutput=True)
        e.dbgert = nc.declare_dram_parameter("dbgert", [1, 128], F32, isOutput=True)
        e.dbgnd = nc.declare_dram_parameter("dbgnd", [128, 68], F32, isOutput=True)

    with ExitStack() as ctx:
        tc = ctx.enter_context(tile.TileContext(nc))

        # ---- shared preamble ----
        pre = ctx.enter_context(tc.tile_pool(name="pre", bufs=1))
        e.ident = pre.tile([128, 128], F32)
        make_identity(nc, e.ident[:])

        e.wq_r = pre.tile([128, 4, 128], F32R)
        e.wk_r = pre.tile([128, 4, 128], F32R)
        e.wv_r = pre.tile([128, 4, 128], F32R)
        for w_sb, w_dr in ((e.wq_r, e.wq), (e.wk_r, e.wk), (e.wv_r, e.wv)):
            nc.sync.dma_start(
                w_sb[:], w_dr[:].bitcast(F32R).rearrange("(c p) f -> p c f", p=128))
        e.wo_r = pre.tile([128, DM], F32R)
        nc.sync.dma_start(e.wo_r[:], e.wo[:].bitcast(F32R))

        pn_f = pre.tile([128, NF + 4], F32)
        nc.sync.dma_start(pn_f[0:DH, 0:NF], e.projT[:])
        nc.gpsimd.memset(pn_f[DH:128, 0:NF], -1.0)
        # col 256 extracts diag (sum of the squared half); 257-259 pad (f32r
        # moving free dim must be a multiple of 4)
        nc.gpsimd.memset(pn_f[0:DH, NF:NF + 4], 0.0)
        nc.gpsimd.memset(pn_f[DH:128, NF:NF + 4], 0.0)
        nc.gpsimd.memset(pn_f[DH:128, NF:NF + 1], 1.0)
        e.projnegP_r = pre.tile([128, NF + 4], F32R)
        nc.vector.tensor_copy(e.projnegP_r[:], pn_f[:])
        e.projneg_r = e.projnegP_r[:, 0:NF]

        ones_f = pre.tile([128, 1], F32)
        nc.gpsimd.memset(ones_f[:], 1.0)
        e.ones_f = ones_f
        e.ones_col_r = pre.tile([128, 1], F32R)
        nc.vector.tensor_copy(e.ones_col_r[:], ones_f[:])
        e.ones_row65 = pre.tile([1, 65], F32)
        nc.gpsimd.memset(e.ones_row65[:], 1.0)
        e.lneps = pre.tile([128, 1], F32)
        nc.gpsimd.memset(e.lneps[:], float(np.log(EPS)))

        # ---- shared residents (used by BOTH branches; SBUF is core-local) ----
        res = ctx.enter_context(tc.tile_pool(name="res", bufs=1))
        e.R1 = res.tile([128, n_tok], F32R)        # global: Aq head0 / local: qTr
        e.R2 = res.tile([128, n_tok], F32R)        # global: Aq head1 / local: kTr
        e.R3 = res.tile([128, NC, 130], F32R)      # v token-major [v0|1|v1|1]
        e.mk_buf = res.tile([128, 2, NC], F32)
        e.vsum_buf = res.tile([128, max(NST, 2)], F32)
        e.ctx_fm = res.tile([128, 2, 2, 68], F32R)  # [p, mchunk, head, col] (68: f32r moving needs %4)
        e.s_row = res.tile([1, 2, 68], F32R)

        # init the ones columns of R3 once (cols 64 and 129 of each chunk)
        for kc in range(NC):
            nc.vector.tensor_copy(e.R3[:, kc, 64:65], e.ones_col_r[:])
            nc.vector.tensor_copy(e.R3[:, kc, 129:130], e.ones_col_r[:])

        # ---- shared pools (tags shared across branches to bound SBUF) ----
        e.ld = ctx.enter_context(tc.tile_pool(name="ld", bufs=2))
        e.wk3 = ctx.enter_context(tc.tile_pool(name="wk3", bufs=3))
        e.wk2 = ctx.enter_context(tc.tile_pool(name="wk2", bufs=2))
        e.psProj = ctx.enter_context(tc.tile_pool(name="psProj", bufs=3, space="PSUM"))
        e.psAcc = ctx.enter_context(tc.tile_pool(name="psAcc", bufs=2, space="PSUM"))
        e.psSm = ctx.enter_context(tc.tile_pool(name="psSm", bufs=3, space="PSUM"))

        pid = nc.partition_id()
        with tc.If(pid < 6) as cmp:
            _global_branch(nc, tc, e)
        with cmp.Else():
            _local_branch(nc, tc, e)

    return nc


def _tr(nc, e, out_ap, in_ap):
    k = in_ap.shape[0]
    nc.tensor.transpose(out_ap, in_ap, e.ident[0:k, 0:k])


def _load_xt(nc, e, t):
    ts = slice(t * e.TT, (t + 1) * e.TT)
    xt = e.ld.tile([128, 4, e.TT], F32R, tag="xt")
    nc.sync.dma_start(
        xt[:], e.xT[:, ts].bitcast(F32R).rearrange("(c p) t -> p c t", p=128))
    return xt


def _project(nc, e, xt, w_r):
    """q/k/v projection into PSUM [128 = 2 heads x 64, TT]."""
    pp = e.psProj.tile([128, e.TT], F32, tag="proj")
    for c in range(4):
        nc.tensor.matmul(pp[:], w_r[:, c, :], xt[:, c, :],
                         start=(c == 0), stop=(c == 3))
    return pp


def _v_tokmajor(nc, e, t, v_ps):
    """v [128, TT] PSUM d-major -> R3 chunks [tok128, v0|1|v1|1]."""
    v_sb = e.wk2.tile([128, e.TT], F32, tag="vsb")
    nc.scalar.activation(v_sb[:], v_ps[:], AF.Identity,
                         accum_out=e.vsum_buf[:, t:t + 1])
    for su in range(4):
        kc = t * 4 + su
        vtr_ps = e.psSm.tile([128, 128], F32, tag="sm")
        _tr(nc, e, vtr_ps[:], v_sb[:, su * 128:(su + 1) * 128])
        nc.vector.tensor_copy(e.R3[:, kc, 0:64], vtr_ps[:, 0:64])
        nc.vector.tensor_copy(e.R3[:, kc, 65:129], vtr_ps[:, 64:128])


def _aug_assemble(nc, e, p_ps, dest0, dest1, ts, tag):
    """[2-head packed PSUM [128,TT]] -> per-head augmented [n*NORM ; n^2*c2]
    written into dest0/dest1 [128, ts]. Lane engines can't shift partitions,
    so the cross-half moves are SBUF->SBUF DMAs."""
    qn = e.wk2.tile([128, e.TT], F32R, tag=f"{tag}n")
    nc.scalar.mul(qn[:], p_ps[:], NORM)
    sq = e.wk2.tile([128, e.TT], F32R, tag=f"{tag}s")
    nc.scalar.activation(sq[:], p_ps[:], AF.Square, scale=SQRT_C2)
    nc.vector.tensor_copy(dest0[0:64, ts], qn[0:64, :])
    nc.sync.dma_start(dest0[64:128, ts], sq[0:64, :])
    nc.sync.dma_start(dest1[0:64, ts], qn[64:128, :])
    nc.vector.tensor_copy(dest1[64:128, ts], sq[64:128, :])


def _global_branch(nc, tc, e):
    NST, NC, TT, n_tok = e.NST, e.NC, e.TT, e.n_tok
    Aq = [e.R1, e.R2]

    # ---------------- phase G1: k/v side + Aq build ----------------
    ctx_ps = []
    for h in range(2):
        acc_t = e.psAcc.tile([65, NF], F32, tag="acc", name=f"acc{h}")
        ctx_ps.append(acc_t)
    for t in range(NST):
        ts = slice(t * TT, (t + 1) * TT)
        xt = _load_xt(nc, e, t)

        q_ps = _project(nc, e, xt, e.wq_r)
        _aug_assemble(nc, e, q_ps, Aq[0], Aq[1], ts, "q")

        k_ps = _project(nc, e, xt, e.wk_r)
        ak0 = e.wk2.tile([128, TT], F32R, tag="ak0")
        ak1 = e.wk2.tile([128, TT], F32R, tag="ak1")
        _aug_assemble(nc, e, k_ps, ak0, ak1, slice(0, TT), "k")
        ak = [ak0, ak1]

        v_ps = _project(nc, e, xt, e.wv_r)
        _v_tokmajor(nc, e, t, v_ps)

        for su in range(4):
            kc = t * 4 + su
            ss = slice(su * 128, (su + 1) * 128)
            for h in range(2):
                zk_ps = e.psSm.tile([128, NF + 4], F32, tag="sm")
                nc.tensor.matmul(zk_ps[:], ak[h][:, ss], e.projnegP_r[:],
                                 start=True, stop=True)
                # reference maxes are over dd = z + diag (diag in col 256)
                zmax = e.wk3.tile([128, 1], F32, tag="zmax")
                nc.vector.reduce_max(zmax[:], zk_ps[:, 0:NF], axis=AX.X)
                nc.vector.tensor_tensor(e.mk_buf[:, h, kc:kc + 1], zmax[:],
                                        zk_ps[:, NF:NF + 1], ALU.add)
                kp = e.wk3.tile([128, NF], F32R, tag="kp")
                nc.scalar.activation(kp[:], zk_ps[:, 0:NF], AF.Exp)
                if e.debug and kc == 0 and h == 0:
                    zk_sb = e.wk3.tile([128, NF], F32, tag="dbg")
                    nc.vector.tensor_copy(zk_sb[:], zk_ps[:, 0:NF])
                    nc.sync.dma_start(e.dbgzk[:], zk_sb[:])
                    nc.sync.dma_start(e.dbgkp[:], kp[:].bitcast(F32))
                nc.tensor.matmul(ctx_ps[h][:], e.R3[:, kc, h * 65:(h + 1) * 65],
                                 kp[:], start=(kc == 0), stop=(kc == NC - 1))

    # ---- k-side fixups ----
    if e.debug:
        nc.sync.dma_start(e.dbgAq[:], Aq[0][:, 0:512].bitcast(F32))
    vsum = e.wk2.tile([128, 1], F32, tag="vsum")
    nc.vector.reduce_sum(vsum[:], e.vsum_buf[:, 0:NST], axis=AX.X)
    vst_ps = e.psSm.tile([1, 128], F32, tag="sm")
    _tr(nc, e, vst_ps[:], vsum[:])
    vsumT = e.wk2.tile([1, 128], F32, tag="vsumT")
    nc.vector.tensor_copy(vsumT[:], vst_ps[:])

    for h in range(2):
        mk_red = e.wk2.tile([128, 1], F32, tag="mkred")
        nc.vector.reduce_max(mk_red[:], e.mk_buf[:, h, :], axis=AX.X)
        mkt_ps = e.psSm.tile([1, 128], F32, tag="sm")
        _tr(nc, e, mkt_ps[:], mk_red[:])
        mkt = e.wk2.tile([1, 128], F32, tag="mkt")
        nc.vector.tensor_copy(mkt[:], mkt_ps[:])
        mk_sc = e.wk2.tile([1, 1], F32, tag="mksc")
        nc.vector.reduce_max(mk_sc[:], mkt[:], axis=AX.X)
        f_sc = e.wk2.tile([1, 1], F32, tag="fsc")
        nc.scalar.activation(f_sc[:], mk_sc[:], AF.Exp, scale=-1.0)
        fb_ps = e.psSm.tile([65, 1], F32, tag="sm")
        nc.tensor.matmul(fb_ps[:], e.ones_row65[:], f_sc[:], start=True, stop=True)
        fb = e.wk2.tile([65, 1], F32, tag="fb")
        nc.vector.tensor_copy(fb[:], fb_ps[:])

        ev_row = e.wk2.tile([1, 65], F32, tag="evrow")
        nc.scalar.mul(ev_row[:, 0:64], vsumT[:, h * DH:(h + 1) * DH], EPS)
        nc.gpsimd.memset(ev_row[:, 64:65], EPS * n_tok)
        ev_ps = e.psSm.tile([65, 1], F32, tag="sm")
        _tr(nc, e, ev_ps[:], ev_row[:])
        epsv = e.wk2.tile([65, 1], F32, tag="epsv")
        nc.vector.tensor_copy(epsv[:], ev_ps[:])

        ctxT = e.wk2.tile([65, NF], F32, tag="ctxT")
        nc.vector.tensor_scalar(ctxT[:], ctx_ps[h][:], fb[:], epsv[:],
                                ALU.mult, ALU.add)
        if e.debug and h == 0:
            ctxraw_sb = e.wk3.tile([65, NF], F32, tag="dbg2")
            nc.vector.tensor_copy(ctxraw_sb[:], ctx_ps[h][:])
            nc.sync.dma_start(e.dbgctx[:], ctxraw_sb[:])
            nc.sync.dma_start(e.dbgctxT[:], ctxT[:])
            nc.sync.dma_start(e.dbgmeta[0:1, 0:1], mk_sc[:])
            nc.sync.dma_start(e.dbgmeta[0:1, 1:2], f_sc[:])
            nc.sync.dma_start(e.dbgmeta[0:1, 2:3], vsum[0:1, :])
            nc.sync.dma_start(e.dbgmeta[0:1, 3:4], epsv[0:1, :])
        for c in range(2):
            cf_ps = e.psSm.tile([128, 65], F32, tag="sm")
            _tr(nc, e, cf_ps[:], ctxT[:, c * 128:(c + 1) * 128])
            nc.vector.tensor_copy(e.ctx_fm[:, c, h, 0:65], cf_ps[:])
            nc.vector.tensor_copy(e.ctx_fm[:, c, h, 65:68], cf_ps[:, 0:3])
        sr_ps = e.psSm.tile([1, 65], F32, tag="sm")
        for c in range(2):
            nc.tensor.matmul(sr_ps[:], e.ones_f[:],
                             e.ctx_fm[:, c, h, 0:65].bitcast(F32),
                             start=(c == 0), stop=(c == 1))
        nc.vector.tensor_copy(e.s_row[:, h, 0:65], sr_ps[:])
        nc.vector.tensor_copy(e.s_row[:, h, 65:68], sr_ps[:, 0:3])
        if e.debug and h == 0:
            nc.sync.dma_start(e.dbgcfm[:], e.ctx_fm[:, :, 0, :].bitcast(F32))
            nc.sync.dma_start(e.dbgsrow[:], e.s_row[:, 0, :].bitcast(F32))

    # ---------------- phase G2: q side ----------------
    for t in range(NST):
        ts = slice(t * TT, (t + 1) * TT)
        qp = [[None, None], [None, None]]
        ert = [None, None]
        for h in range(2):
            for c in range(2):
                zf_ps = e.psProj.tile([128, TT], F32, tag="proj")
                nc.tensor.matmul(zf_ps[:], e.projneg_r[:, c * 128:(c + 1) * 128],
                                 Aq[h][:, ts], start=True, stop=True)
                qp_c = e.wk2.tile([128, TT], F32R, tag=f"qp{h}{c}")
                nc.scalar.activation(qp_c[:], zf_ps[:], AF.Exp)
                qp[h][c] = qp_c
                if e.debug and t == 0 and h == 0 and c == 0:
                    nc.sync.dma_start(e.dbgqp[:], qp_c[:].bitcast(F32))
            ert_h = []
            for su in range(4):
                zt_ps = e.psSm.tile([128, NF + 4], F32, tag="sm")
                nc.tensor.matmul(
                    zt_ps[:], Aq[h][:, t * TT + su * 128: t * TT + (su + 1) * 128],
                    e.projnegP_r[:], start=True, stop=True)
                zmax = e.wk3.tile([128, 1], F32, tag="zmax")
                nc.vector.reduce_max(zmax[:], zt_ps[:, 0:NF], axis=AX.X)
                mq = e.wk3.tile([128, 1], F32, tag="mq")
                nc.vector.tensor_tensor(mq[:], zmax[:], zt_ps[:, NF:NF + 1], ALU.add)
                er = e.wk3.tile([128, 1], F32, tag="er")
                nc.scalar.activation(er[:], mq[:], AF.Exp, bias=e.lneps[:])
                ert_ps = e.psSm.tile([1, 128], F32, tag="sm")
                _tr(nc, e, ert_ps[:], er[:])
                ert_su = e.wk3.tile([1, 128], F32R, tag=f"ert{h}", name=f"ert{h}_{su}")
                nc.vector.tensor_copy(ert_su[:], ert_ps[:])
                ert_h.append(ert_su)
                if e.debug and t == 0 and h == 0 and su == 0:
                    nc.sync.dma_start(e.dbgert[:], ert_su[:].bitcast(F32))
            ert[h] = ert_h

        for su in range(4):
            ss = slice(su * 128, (su + 1) * 128)
            row0 = t * TT + su * 128
            ao = e.wk3.tile([128, 128], F32, tag="ao")
            for h in range(2):
                nd_ps = e.psSm.tile([128, 68], F32, tag="sm")
                nc.tensor.matmul(nd_ps[:], qp[h][0][:, ss], e.ctx_fm[:, 0, h, :],
                                 start=True, stop=False)
                nc.tensor.matmul(nd_ps[:], qp[h][1][:, ss], e.ctx_fm[:, 1, h, :],
                                 start=False, stop=False)
                nc.tensor.matmul(nd_ps[:], ert[h][su][:], e.s_row[:, h, :],
                                 start=False, stop=True)
                if e.debug and t == 0 and su == 0 and h == 0:
                    nd_sb = e.wk3.tile([128, 68], F32, tag="dbg3")
                    nc.vector.tensor_copy(nd_sb[:], nd_ps[:])
                    nc.sync.dma_start(e.dbgnd[:], nd_sb[:])
                rec = e.wk3.tile([128, 1], F32, tag="rec")
                nc.vector.reciprocal(rec[:], nd_ps[:, 64:65])
                nc.vector.tensor_scalar_mul(ao[:, h * DH:(h + 1) * DH],
                                            nd_ps[:, 0:64], rec[:])
            _project_out(nc, e, ao, row0)


def _project_out(nc, e, ao, row0):
    """attn-out token-major [128,128] -> transpose -> y rows via Wo slice."""
    aoT_ps = e.psSm.tile([128, 128], F32, tag="sm")
    _tr(nc, e, aoT_ps[:], ao[:])
    aoT = e.wk3.tile([128, 128], F32R, tag="aoTs")
    nc.vector.tensor_copy(aoT[:], aoT_ps[:])
    y_ps = e.psProj.tile([128, DM], F32, tag="proj")
    nc.tensor.matmul(y_ps[:], aoT[:], e.wo_r[:], start=True, stop=True)
    y_sb = e.wk2.tile([128, DM], F32, tag="ysb")
    nc.scalar.copy(y_sb[:], y_ps[:])
    nc.sync.dma_start(e.y[row0:row0 + 128, :], y_sb[:])


def _local_branch(nc, tc, e):
    NST, NC, TT, NW = e.NST, e.NC, e.TT, e.NW
    qTr, kTr = e.R1, e.R2

    # ---------------- phase L1: projections + rotary ----------------
    for t in range(NST):
        ts = slice(t * TT, (t + 1) * TT)
        xt = _load_xt(nc, e, t)
        cos2 = e.ld.tile([128, TT], F32, tag="cos2")
        nc.sync.dma_start(cos2[0:DH, :], e.cosT[:, ts])
        nc.sync.dma_start(cos2[DH:128, :], cos2[0:DH, :])
        sin2 = e.ld.tile([128, TT], F32, tag="sin2")
        nc.sync.dma_start(sin2[0:DH, :], e.sinNT[:, ts])
        nc.sync.dma_start(sin2[DH:128, :], sin2[0:DH, :])

        for w_r, dest in ((e.wq_r, qTr), (e.wk_r, kTr)):
            pp = _project(nc, e, xt, w_r)
            p_sb = e.wk2.tile([128, TT], F32, tag="qn")
            nc.vector.tensor_copy(p_sb[:], pp[:])
            p_sw = e.wk2.tile([128, TT], F32, tag="qs")
            for h in range(2):
                o = h * DH
                nc.sync.dma_start(p_sw[o:o + 32, :], p_sb[o + 32:o + 64, :])
                nc.sync.dma_start(p_sw[o + 32:o + 64, :], p_sb[o:o + 32, :])
            t1 = e.wk2.tile([128, TT], F32, tag="kn")
            nc.vector.tensor_tensor(t1[:], p_sb[:], cos2[:], ALU.mult)
            t2 = e.wk2.tile([128, TT], F32, tag="ks")
            nc.vector.tensor_tensor(t2[:], p_sw[:], sin2[:], ALU.mult)
            nc.vector.tensor_tensor(dest[:, ts], t1[:], t2[:], ALU.add)

        v_ps = _project(nc, e, xt, e.wv_r)
        _v_tokmajor(nc, e, t, v_ps)

    # ---------------- phase L2: windowed attention ----------------
    # Windows processed in PAIRS: adjacent windows share k-chunks, so one
    # [128, 512] dots matmul + one exp covers both windows per k-chunk
    # (halves the ACT op count and the dots matmul count).
    for wp in range(NW // 2):
        wA, wB = 2 * wp, 2 * wp + 1
        qs = slice(wA * WIN, (wB + 1) * WIN)          # 512 queries
        cA0, cA1 = max(0, 2 * wA - 2), min(NC - 1, 2 * wA + 3)
        cB0, cB1 = max(0, 2 * wB - 2), min(NC - 1, 2 * wB + 3)
        olT_all = {}
        for h in range(2):
            hs = slice(h * DH, (h + 1) * DH)
            olA = e.psAcc.tile([65, WIN], F32, tag="acc", name=f"olA_{h}")
            olB = e.psAcc.tile([65, WIN], F32, tag="acc", name=f"olB_{h}")
            for kc in range(cA0, cB1 + 1):
                dk_ps = e.psProj.tile([128, 2 * WIN], F32, tag="proj")
                nc.tensor.matmul(dk_ps[:], kTr[hs, kc * 128:(kc + 1) * 128],
                                 qTr[hs, qs], start=True, stop=True)
                P = e.wk3.tile([128, 2 * WIN], F32R, tag="P2")
                nc.scalar.activation(P[:], dk_ps[:], AF.Exp, scale=0.125)
                if cA0 <= kc <= cA1:
                    nc.tensor.matmul(olA[:], e.R3[:, kc, h * 65:(h + 1) * 65],
                                     P[:, 0:WIN], start=(kc == cA0),
                                     stop=(kc == cA1))
                if cB0 <= kc <= cB1:
                    nc.tensor.matmul(olB[:], e.R3[:, kc, h * 65:(h + 1) * 65],
                                     P[:, WIN:2 * WIN], start=(kc == cB0),
                                     stop=(kc == cB1))
            for w, olp in ((wA, olA), (wB, olB)):
                olT_h = e.wk3.tile([65, WIN], F32, tag="olT", bufs=4,
                                   name=f"olT{w}_{h}")
                nc.vector.tensor_copy(olT_h[:], olp[:])
                olT_all[(h, w)] = olT_h
        for w in (wA, wB):
            olT = [olT_all[(0, w)], olT_all[(1, w)]]
            for su in range(2):
                row0 = w * WIN + su * 128
                ao = e.wk3.tile([128, 128], F32, tag="ao")
                for h in range(2):
                    tr_ps = e.psSm.tile([128, 65], F32, tag="sm")
                    _tr(nc, e, tr_ps[:], olT[h][:, su * 128:(su + 1) * 128])
                    rec = e.wk3.tile([128, 1], F32, tag="rec")
                    nc.vector.reciprocal(rec[:], tr_ps[:, 64:65])
                    nc.vector.tensor_scalar_mul(ao[:, h * DH:(h + 1) * DH],
                                                tr_ps[:, 0:64], rec[:])
                _project_out(nc, e, ao, row0)


# ---------------------------------------------------------------------------
# host wrapper
# ---------------------------------------------------------------------------
_PROG_CACHE = {}


def _get_program(n_tok: int):
    if n_tok not in _PROG_CACHE:
        _install_legalizer()
        _PROG_CACHE[n_tok] = build_program(n_tok)
    return _PROG_CACHE[n_tok]


def _rotary_tables(n_tok: int):
    inv_freq = 1.0 / (10000.0 ** (np.arange(0, DH, 2, dtype=np.float32) / DH))
    t = np.arange(n_tok, dtype=np.float32)
    freqs = t[:, None] * inv_freq[None, :]
    freqs = np.concatenate([freqs, freqs], axis=-1)
    cos = np.cos(freqs).T.astype(np.float32)
    sin = np.sin(freqs).T.astype(np.float32)
    sinN = np.concatenate([-sin[0:32], sin[32:64]], axis=0)
    return np.ascontiguousarray(cos), np.ascontiguousarray(sinN)


def make_in_maps(x, Wq, Wk, Wv, Wo, proj):
    B, n_tok, _ = x.shape
    cos, sinN = _rotary_tables(n_tok)
    projT = np.ascontiguousarray(proj.T)
    xTs = [np.ascontiguousarray(x[b].T) for b in range(B)]
    in_maps = []
    for c in range(8):
        if c < 6:
            b, hp = c // 3, c % 3
        else:
            b, hp = c - 6, 3
        cs = slice(hp * 128, (hp + 1) * 128)
        in_maps.append({
            "xT": xTs[b],
            "wq": np.ascontiguousarray(Wq[:, cs]),
            "wk": np.ascontiguousarray(Wk[:, cs]),
            "wv": np.ascontiguousarray(Wv[:, cs]),
            "wo": np.ascontiguousarray(Wo[cs, :]),
            "projT": projT,
            "cosT": cos,
            "sinNT": sinN,
        })
    return in_maps


def combine_outputs(parts, bo, B, n_tok):
    out = np.empty((B, n_tok, DM), np.float32)
    out[0] = parts[0] + parts[1] + parts[2] + parts[6] + bo
    out[1] = parts[3] + parts[4] + parts[5] + parts[7] + bo
    return out


def kernel(x, Wq, Wk, Wv, Wo, bo, proj):
    x = np.asarray(x, np.float32)
    Wq, Wk, Wv, Wo = (np.asarray(a, np.float32) for a in (Wq, Wk, Wv, Wo))
    bo = np.asarray(bo, np.float32)
    proj = np.asarray(proj, np.float32)
    B, n_tok, _ = x.shape
    assert B == 2

    nc = _get_program(n_tok)
    in_maps = make_in_maps(x, Wq, Wk, Wv, Wo, proj)
    res = run_bass_kernel_spmd(nc, in_maps, list(range(8)))
    parts = [res.results[c]["y"] for c in range(8)]
    return combine_outputs(parts, bo, B, n_tok)

